# revision 2
# baseline (speedup 1.0000x reference)
"""LSTMCell (B=4096, IN=2048, H=2048, fp32) on 8 Trainium2 NeuronCores.

Tensor-parallel: hidden dim sharded 8 ways (256 rows of each gate weight per
core). input/h0 replicated, c0/outputs sharded. Per core, the 4 gate
pre-activations are computed as pre.T = W_shard @ [x, h0].T via PE matmuls
(fp16 operands, fp32 PSUM accumulation), with bias-add + tanh/sigmoid fused
into the PSUM->SBUF eviction on the scalar engine, and the cell/hidden
elementwise math on the vector engine. Outputs are produced transposed
([h, b]) and reassembled on the host.
"""
import sys

if '/opt/trn_rl_repo' not in sys.path:
    sys.path.insert(0, '/opt/trn_rl_repo')

import numpy as np

import concourse.bass as bass  # noqa: F401  (registers AP machinery)
import concourse.mybir as mybir
import concourse.tile as tile
from concourse import bacc
from concourse.bass_utils import run_bass_kernel_spmd

B, IN, H = 4096, 2048, 2048
NCORES = 8
HSH = H // NCORES          # 256 hidden rows per core
NG = 4 * HSH               # 1024 gate rows per core (g,i,f,o)
K = IN + H                 # 4096 contraction
KT = K // 128              # 32 k-tiles
HT = NG // 128             # 8 h-tiles per core
BC = 512                   # batch chunk (psum free dim)
NBC = B // BC              # 8 batch chunks
DT = mybir.dt.float16
NPDT = np.float16

_cache = {}


def _build():
    nc = bacc.Bacc("TRN2", target_bir_lowering=False, debug=False)
    names = {}
    with tile.TileContext(nc) as tc:
        with tc.tile_pool(name="dram", bufs=1, space="DRAM") as dram, \
             tc.tile_pool(name="wpool", bufs=1) as wpool, \
             tc.tile_pool(name="xpool", bufs=2) as xpool, \
             tc.tile_pool(name="apool", bufs=12) as apool, \
             tc.tile_pool(name="cpool", bufs=3) as cpool, \
             tc.tile_pool(name="epool", bufs=2) as epool, \
             tc.tile_pool(name="psum", bufs=4, space="PSUM") as psum:

            # DRAM I/O. Host pre-tiles everything partition-major so every
            # DMA is 128 fully-contiguous per-partition runs.
            # xh4: [128p, NBC*KT*BC] = xhT reshaped (p, bc, k, b)
            xh_d = dram.tile([128, NBC * KT * BC], DT, kind="ExternalInput")
            # w4: [128p, KT*NG] = wT reshaped (p, k, h)
            w_d = dram.tile([128, KT * NG], DT, kind="ExternalInput")
            # bias: [128p, HT]
            b_d = dram.tile([128, HT], mybir.dt.float32, kind="ExternalInput")
            # c0t: [2*128, B] (hidden-shard-major transpose of c0 slice)
            c_d = dram.tile([2 * 128, B], mybir.dt.float32, kind="ExternalInput")
            cy_d = dram.tile([2 * 128, B], mybir.dt.float32, kind="ExternalOutput")
            hy_d = dram.tile([2 * 128, B], mybir.dt.float32, kind="ExternalOutput")
            names = dict(xh=xh_d.name, w=w_d.name, bias=b_d.name, c0=c_d.name,
                         cy=cy_d.name, hy=hy_d.name)

            w_sb = wpool.tile([128, KT * NG], DT)       # 64KB/p, resident
            nc.sync.dma_start(w_sb[:], w_d[:])
            bias_sb = wpool.tile([128, HT], mybir.dt.float32)
            nc.sync.dma_start(bias_sb[:], b_d[:])

            ACT = mybir.ActivationFunctionType
            gate_fn = [ACT.Tanh, ACT.Sigmoid, ACT.Sigmoid, ACT.Sigmoid]

            for bc in range(NBC):
                xh_sb = xpool.tile([128, KT * BC], DT)  # 32KB/p
                nc.sync.dma_start(
                    xh_sb[:], xh_d[:, bc * KT * BC:(bc + 1) * KT * BC])

                acts = []
                for ht in range(HT):
                    pt = psum.tile([128, BC], mybir.dt.float32, tag="pt")
                    for k in range(KT):
                        nc.tensor.matmul(
                            pt[:],
                            w_sb[:, k * NG + ht * 128: k * NG + (ht + 1) * 128],
                            xh_sb[:, k * BC:(k + 1) * BC],
                            start=(k == 0), stop=(k == KT - 1),
                        )
                    a = apool.tile([128, BC], mybir.dt.float32, tag="acts")
                    nc.scalar.activation(a[:], pt[:], gate_fn[ht // 2],
                                         bias=bias_sb[:, ht:ht + 1])
                    acts.append(a)

                for hh in range(2):
                    g_t, i_t, f_t, o_t = (acts[gi * 2 + hh] for gi in range(4))
                    c0_t = cpool.tile([128, BC], mybir.dt.float32, tag="c0")
                    nc.sync.dma_start(
                        c0_t[:], c_d[hh * 128:(hh + 1) * 128,
                                     bc * BC:(bc + 1) * BC])
                    ig = epool.tile([128, BC], mybir.dt.float32, tag="ig")
                    nc.vector.tensor_mul(ig[:], i_t[:], g_t[:])
                    fc = epool.tile([128, BC], mybir.dt.float32, tag="fc")
                    nc.vector.tensor_mul(fc[:], f_t[:], c0_t[:])
                    cy = epool.tile([128, BC], mybir.dt.float32, tag="cy")
                    nc.vector.tensor_add(cy[:], ig[:], fc[:])
                    tcy = epool.tile([128, BC], mybir.dt.float32, tag="tcy")
                    nc.scalar.activation(tcy[:], cy[:], ACT.Tanh)
                    hy = epool.tile([128, BC], mybir.dt.float32, tag="hy")
                    nc.vector.tensor_mul(hy[:], o_t[:], tcy[:])
                    nc.sync.dma_start(
                        cy_d[hh * 128:(hh + 1) * 128, bc * BC:(bc + 1) * BC],
                        cy[:])
                    nc.sync.dma_start(
                        hy_d[hh * 128:(hh + 1) * 128, bc * BC:(bc + 1) * BC],
                        hy[:])
    nc.compile()
    return nc, names


def _make_in_maps(inputs, names):
    input, h0, c0 = inputs['input'], inputs['h0'], inputs['c0']
    # xhT [K, B] -> pre-tiled (p, bc, k, b) -> [128, NBC*KT*BC], fp16
    xht = np.concatenate([np.asarray(input), np.asarray(h0)], axis=1).T
    xh4 = np.ascontiguousarray(
        xht.reshape(KT, 128, NBC, BC).transpose(1, 2, 0, 3)
    ).astype(NPDT).reshape(128, NBC * KT * BC)

    in_maps = []
    for c in range(NCORES):
        hs = slice(c * HSH, (c + 1) * HSH)
        wx = np.concatenate(
            [np.asarray(inputs[w])[hs] for w in ('Wgx', 'Wix', 'Wfx', 'Wox')], axis=0)
        wh = np.concatenate(
            [np.asarray(inputs[w])[hs] for w in ('Wgh', 'Wih', 'Wfh', 'Woh')], axis=0)
        wt = np.concatenate([wx, wh], axis=1).T            # [K, NG]
        w4 = np.ascontiguousarray(
            wt.reshape(KT, 128, NG).transpose(1, 0, 2)
        ).astype(NPDT).reshape(128, KT * NG)
        bias = np.concatenate([
            np.asarray(inputs[bx])[hs] + np.asarray(inputs[bh])[hs]
            for bx, bh in (('bgx', 'bgh'), ('bix', 'bih'),
                           ('bfx', 'bfh'), ('box', 'boh'))
        ]).astype(np.float32)                              # [NG]
        bias_g = np.ascontiguousarray(bias.reshape(HT, 128).T)  # [128, HT]
        c0t = np.ascontiguousarray(np.asarray(c0)[:, hs].T.astype(np.float32))
        in_maps.append({
            names['xh']: xh4,
            names['w']: w4,
            names['bias']: bias_g,
            names['c0']: c0t,
        })
    return in_maps


def kernel(input, h0, c0, Wgx, bgx, Wgh, bgh, Wix, bix, Wih, bih,
           Wfx, bfx, Wfh, bfh, Wox, box, Woh, boh):
    if 'nc' not in _cache:
        _cache['nc'], _cache['names'] = _build()
    nc, names = _cache['nc'], _cache['names']

    inputs = dict(input=input, h0=h0, c0=c0, Wgx=Wgx, bgx=bgx, Wgh=Wgh,
                  bgh=bgh, Wix=Wix, bix=bix, Wih=Wih, bih=bih, Wfx=Wfx,
                  bfx=bfx, Wfh=Wfh, bfh=bfh, Wox=Wox, box=box, Woh=Woh,
                  boh=boh)
    in_maps = _make_in_maps(inputs, names)

    res = run_bass_kernel_spmd(nc, in_maps, core_ids=list(range(NCORES)))
    cy = np.concatenate([r[names['cy']] for r in res.results], axis=0).T
    hy = np.concatenate([r[names['hy']] for r in res.results], axis=0).T
    return (np.ascontiguousarray(hy, dtype=np.float32),
            np.ascontiguousarray(cy, dtype=np.float32))
